# revision 4
# baseline (speedup 1.0000x reference)
"""DenseCapsule dynamic-routing kernel v2 for 8 Trainium2 NeuronCores.

Problem: x [256,1152,8] f32, weight [10,1152,16,8] f32 ->
  x_hat = einsum('oidc,bic->boid', weight, x)
  3 rounds of routing-by-agreement (softmax over o, squash over d)
  output [256, 10, 16] f32.

Layout (batch-parallel, 32 samples/core), i = kk*4 + g with g in [0,4):
  - Partition dim p = (b, g) = b*4+g (128 partitions).
  - Phase 1 matmul per kk (all at partition base 0, contraction 32):
      stationary XS[kk][(g,c), (b,g')] = delta_{g,g'} x[b, 4kk+g, c]
      moving     WV[kk][(g,c), (o,d)]  = W[o, 4kk+g, d, c]
    -> PSUM [(b,g), (o,d)] = x_hat[b, o, 4kk+g, d].
    Weight is NOT inflated (2.95 MB); x is inflated 4x (2.36 MB).
  - XH SBUF tile [128, KK, O, 8, 2] fp16 (d split 8x2 so the e-broadcast
    in the m-pass keeps unit innermost stride => DVE 2x mode).
  - t=0 capsule sums: PE-accumulated S1 matmuls over XH kk-triplets
    (S1 = delta_{b,b'} sums g within b, replicated over g').
  - Routing per t in kk-chunks: z = XH*vsum (1 TT), L = sum_d z
    (fp16 reduce), e = exp(L) on ACT engine written twice (pairs),
    Z = tree-sum over o, e' = e * (1/Z), m = XH*e' (per-o TT,
    pair-broadcast), s = S1-supers over m (480-col moving, LDW hidden).
  - squash entirely on [*, O, 8, 2] tiles (d in free dim, no PE).
"""

import sys

for _p in ("/opt/trn_rl_repo",):
    if _p not in sys.path:
        sys.path.insert(0, _p)

import numpy as np

B, I, DIN, O, DOUT = 256, 1152, 8, 10, 16
NCORES = 8
BL = B // NCORES          # 32 samples per core
G = 4                     # i's per phase-1 contraction block
KK = I // G               # 288 kk blocks
KC = 48                   # kk per routing chunk
NJ = (I * DIN) // 128     # 72 s0 chunks of 128 (i,c) rows
NCH = KK // KC            # routing chunks per iteration
TR = 3                    # kk per s-reduce matmul (3*160=480 <= 512)
ND = 12                   # input DMA chunks
KD = KK // ND             # kk per DMA chunk
EPS = 1e-8

_CACHE = {}


def _build_host_constants(weight):
    w5 = weight.reshape(O, KK, G, DOUT, DIN)           # [o,kk,g,d,c]
    wkgc = w5.transpose(1, 2, 4, 0, 3)                 # [kk,g,c,o,d]
    # wv[(g,c), kk, (o,d)] -> [32, KK, 160]
    wv = np.ascontiguousarray(
        wkgc.reshape(KK, 32, O * DOUT).transpose(1, 0, 2)).astype(np.float16)
    # wvj[(i,c) % 128, j, (o,d)] -> [128, NJ, 160]
    wvj = np.ascontiguousarray(
        wkgc.reshape(NJ, 128, O * DOUT).transpose(1, 0, 2)).astype(np.float16)

    bi = np.arange(128) // G
    s1 = (bi[:, None] == bi[None, :]).astype(np.float16)     # [128,128]
    s1f = (bi[:, None] == np.arange(BL)[None, :]).astype(np.float16)
    return wv, wvj, s1, s1f


def _per_core_inputs(xl, wv, wvj, s1, s1f):
    """xl: [BL, I, DIN] fp32 slice for this core."""
    x4 = xl.reshape(BL, KK, G, DIN).astype(np.float16)  # [b,kk,g,c]
    # xs[kk, (g,c), (b,g')] = delta_{g,g'} x[b, 4kk+g, c]
    xs = np.zeros((KK, G, DIN, BL, G), np.float16)      # [kk,g,c,b,g']
    xkcb = x4.transpose(1, 2, 3, 0)                     # [kk,g,c,b]
    for g in range(G):
        xs[:, g, :, :, g] = xkcb[:, g, :, :]
    xs = np.ascontiguousarray(
        xs.reshape(KK, 32, 128).transpose(1, 0, 2))     # [32, KK, 128]

    # xt4[(i,c), (b,g)] = x[b, i, c]
    xt = xl.reshape(BL, I * DIN).T.astype(np.float16)   # [(i,c), b]
    xt4 = np.repeat(xt, G, axis=1)                      # [(i,c), (b,g)]
    xt4 = np.ascontiguousarray(
        xt4.reshape(NJ, 128, 128).transpose(1, 0, 2))   # [128, NJ, 128]
    return {"xs": xs, "wv": wv, "wvj": wvj, "s1": s1, "s1f": s1f,
            "xt4": xt4}


def _squash(nc, small, s_ap, scale, f32, AX, ALU, NP):
    """squash(s*scale) on [NP, O, 8, 2] fp32; returns fp32 tile."""
    s_sb = small.tile([NP, O, 8, 2], f32, tag=f"sq_s{NP}")
    nc.scalar.mul(out=s_sb[:], in_=s_ap, mul=float(scale))
    sq = small.tile([NP, O, 8, 2], f32, tag=f"sq_sq{NP}")
    nc.vector.tensor_mul(sq[:], s_sb[:], s_sb[:])
    m2 = small.tile([NP, O, 1, 1], f32, tag=f"sq_m2{NP}")
    nc.vector.tensor_reduce(out=m2[:], in_=sq[:], axis=AX.XY, op=ALU.add)
    rt = small.tile([NP, O, 1, 1], f32, tag=f"sq_rt{NP}")
    nc.scalar.sqrt(out=rt[:], in_=m2[:])            # sqrt(mag2)
    nc.vector.tensor_scalar_add(rt[:], rt[:], EPS)
    den = small.tile([NP, O, 1, 1], f32, tag=f"sq_den{NP}")
    nc.scalar.add(out=den[:], in_=m2[:], add=1.0)   # 1 + mag2
    nc.vector.tensor_mul(den[:], den[:], rt[:])
    nc.vector.reciprocal_approx_fast(out=den[:, :, 0, 0], in_=den[:, :, 0, 0])
    fac = small.tile([NP, O, 1, 1], f32, tag=f"sq_fac{NP}")
    nc.vector.tensor_mul(fac[:], m2[:], den[:])
    v = small.tile([NP, O, 8, 2], f32, tag=f"sq_v{NP}")
    nc.vector.tensor_mul(v[:], s_sb[:], fac[:].broadcast_to((NP, O, 8, 2)))
    return v


def _build_program():
    import concourse.tile as tile
    from concourse import bacc, mybir

    f16 = mybir.dt.float16
    f32 = mybir.dt.float32
    AF = mybir.ActivationFunctionType
    AX = mybir.AxisListType
    ALU = mybir.AluOpType

    nc = bacc.Bacc(
        "TRN2",
        target_bir_lowering=False,
        debug=False,
        enable_asserts=False,
        num_devices=NCORES,
    )

    xs_d = nc.dram_tensor("xs", [32, KK, 128], f16, kind="ExternalInput")
    wv_d = nc.dram_tensor("wv", [32, KK, O * DOUT], f16, kind="ExternalInput")
    wvj_d = nc.dram_tensor("wvj", [128, NJ, O * DOUT], f16, kind="ExternalInput")
    xt4_d = nc.dram_tensor("xt4", [128, NJ, 128], f16, kind="ExternalInput")
    s1_d = nc.dram_tensor("s1", [128, 128], f16, kind="ExternalInput")
    s1f_d = nc.dram_tensor("s1f", [128, BL], f16, kind="ExternalInput")
    out_d = nc.dram_tensor("out", [BL, O, 8, 2], f32, kind="ExternalOutput")

    with tile.TileContext(nc) as tc:
        with (
            tc.tile_pool(name="const", bufs=1) as const,
            tc.tile_pool(name="xhp", bufs=1) as xhp,
            tc.tile_pool(name="acc", bufs=1) as acc,
            tc.tile_pool(name="small", bufs=1) as small,
            tc.tile_pool(name="zmp", bufs=3) as zmp,
            tc.tile_pool(name="spsum", bufs=1, space="PSUM") as spsum,
        ):
            s1_sb = const.tile([128, 128], f16)
            nc.gpsimd.dma_start(out=s1_sb[:], in_=s1_d.ap())
            s1f_sb = const.tile([128, BL], f16)
            nc.gpsimd.dma_start(out=s1f_sb[:], in_=s1f_d.ap())

            # x_hat, p=(b,g), free (kk, o, dd, r) with d = dd*2+r
            xh = xhp.tile([128, KK, O, 8, 2], f16)
            vsumh = acc.tile([128, O, 8, 2], f16)
            vsum = acc.tile([128, O, 8, 2], f32)

            # ---- Phase 1: x_hat + direct t=0 sums -----------------------
            JD = NJ // ND
            s0sp = spsum.tile([128, O, 8, 2], f32, tag="sp128")
            with (
                tc.tile_pool(name="wpool", bufs=2) as wpool,
                tc.tile_pool(name="xspool", bufs=2) as xspool,
                tc.tile_pool(name="wjpool", bufs=2) as wjpool,
                tc.tile_pool(name="xtpool", bufs=2) as xtpool,
                tc.tile_pool(name="ppsum", bufs=4, space="PSUM") as ppsum,
            ):
                NDS = 4                # dc's carrying the s0 work
                JD2 = NJ // NDS
                for dc in range(ND):
                    wck = wpool.tile([32, KD, O * DOUT], f16)
                    nc.gpsimd.dma_start(
                        out=wck[:], in_=wv_d.ap()[:, dc * KD:(dc + 1) * KD])
                    xsk = xspool.tile([32, KD, 128], f16)
                    nc.gpsimd.dma_start(
                        out=xsk[:], in_=xs_d.ap()[:, dc * KD:(dc + 1) * KD])
                    if dc < NDS:
                        for sub in range(JD2 // JD):
                            j0 = dc * JD2 + sub * JD
                            wjk = wjpool.tile([128, JD, O * DOUT], f16)
                            nc.gpsimd.dma_start(
                                out=wjk[:], in_=wvj_d.ap()[:, j0:j0 + JD])
                            xtk = xtpool.tile([128, JD, 128], f16)
                            nc.gpsimd.dma_start(
                                out=xtk[:], in_=xt4_d.ap()[:, j0:j0 + JD])
                            for jj in range(JD):
                                j = j0 + jj
                                # s0 = sum_(i,c) x*W : accumulate over j
                                nc.tensor.matmul(
                                    s0sp[:],
                                    lhsT=xtk[:, jj, :],
                                    rhs=wjk[:, jj, :],
                                    start=(j == 0),
                                    stop=(j == NJ - 1),
                                )
                    for s in range(KD // TR):
                        pt = ppsum.tile([128, TR, O, 8, 2], f32)
                        for r in range(TR):
                            nc.tensor.matmul(
                                pt[:, r],
                                lhsT=xsk[:, s * TR + r, :],
                                rhs=wck[:, s * TR + r, :],
                                start=True,
                                stop=True,
                            )
                        kk0 = dc * KD + s * TR
                        dst = xh[:, kk0:kk0 + TR]
                        if s % 6 == 0:
                            nc.vector.tensor_copy(out=dst, in_=pt[:])
                        else:
                            nc.scalar.copy(out=dst, in_=pt[:])
                    if dc == NDS - 1:
                        # ---- t = 0: uniform c = 1/10 (early) ------------
                        v = _squash(nc, small, s0sp[:], 1.0 / O,
                                    f32, AX, ALU, 128)
                        nc.vector.tensor_copy(out=vsum[:], in_=v[:])
                        nc.scalar.copy(out=vsumh[:], in_=vsum[:])

            # ---- t = 1, 2 ------------------------------------------------
            with nc.allow_low_precision(reason="logits/softmax in fp16"):
                for t in (1, 2):
                    final = t == 2
                    sS = s1f_sb if final else s1_sb
                    NP = BL if final else 128
                    sp = spsum.tile([NP, TR, O, 8, 2], f32, tag=f"tsp{NP}")
                    for ch in range(NCH):
                        k0 = ch * KC
                        zm = zmp.tile([128, KC, O, 8, 2], f16, tag="zm")
                        # z = XH * vsum  (vsum bcast over kk)
                        nc.vector.tensor_mul(
                            zm[:],
                            xh[:, k0:k0 + KC],
                            vsumh[:].unsqueeze(1)
                            .broadcast_to((128, KC, O, 8, 2)),
                        )
                        # L = sum_d z -> [128, KC, O] fp16 (in-place pair
                        # tree in zm; all levels unit-stride => DVE 2x)
                        nc.vector.tensor_add(
                            zm[:, :, :, 0:4], zm[:, :, :, 0:4], zm[:, :, :, 4:8])
                        nc.vector.tensor_add(
                            zm[:, :, :, 0:2], zm[:, :, :, 0:2], zm[:, :, :, 2:4])
                        nc.vector.tensor_add(
                            zm[:, :, :, 0:1], zm[:, :, :, 0:1], zm[:, :, :, 1:2])
                        L = small.tile([128, KC, O], f16, tag=f"L{ch % 2}")
                        nc.vector.tensor_add(
                            L[:], zm[:, :, :, 0, 0], zm[:, :, :, 0, 1])
                        # e = exp(L), written twice (pairs)
                        e2 = small.tile([128, KC, O, 2], f16, tag=f"e2{ch % 2}")
                        nc.scalar.activation(
                            out=e2[:, :, :, 0], in_=L[:], func=AF.Exp)
                        nc.scalar.activation(
                            out=e2[:, :, :, 1], in_=L[:], func=AF.Exp)
                        # Z = sum_o e (pairs tree)
                        t5 = small.tile([128, KC, 5, 2], f16, tag="t5")
                        nc.vector.tensor_add(t5[:], e2[:, :, 0:5], e2[:, :, 5:10])
                        u2 = small.tile([128, KC, 2, 2], f16, tag="u2")
                        nc.vector.tensor_add(u2[:], t5[:, :, 0:2], t5[:, :, 2:4])
                        zden = small.tile([128, KC, 1, 2], f32, tag="zden")
                        nc.vector.tensor_add(zden[:], u2[:, :, 0:1], u2[:, :, 1:2])
                        nc.vector.tensor_add(zden[:], zden[:], t5[:, :, 4:5])
                        nc.vector.reciprocal_approx_fast(
                            out=zden[:, :, 0, :], in_=zden[:, :, 0, :])
                        zinv = small.tile([128, KC, 1, 2], f16, tag="zinv")
                        nc.vector.tensor_copy(out=zinv[:], in_=zden[:])
                        # e' = e * (1/Z)  (bcast over o)
                        nc.vector.tensor_mul(
                            e2[:], e2[:], zinv[:].broadcast_to((128, KC, O, 2)))
                        # m = XH * e'  (pair-bcast over dd) -- per o
                        for o in range(O):
                            nc.vector.tensor_mul(
                                zm[:, :, o],
                                xh[:, k0:k0 + KC, o],
                                e2[:, :, o].unsqueeze(2)
                                .broadcast_to((128, KC, 8, 2)),
                            )
                        # s += sum_{kk,g} m : PE accumulation, kk-triplets
                        for s in range(KC // TR):
                            nc.tensor.matmul(
                                sp[:],
                                lhsT=sS[:],
                                rhs=zm[:, TR * s:TR * s + TR],
                                start=(ch == 0 and s == 0),
                                stop=(ch == NCH - 1 and s == KC // TR - 1),
                            )
                    stot = small.tile([NP, O, 8, 2], f32, tag=f"stot{NP}")
                    nc.scalar.copy(out=stot[:], in_=sp[:, 0])
                    nc.vector.tensor_add(stot[:], stot[:], sp[:, 1])
                    nc.vector.tensor_add(stot[:], stot[:], sp[:, 2])
                    v = _squash(nc, small, stot[:], 1.0, f32, AX, ALU, NP)
                    if final:
                        nc.gpsimd.dma_start(out=out_d.ap(), in_=v[:])
                    else:
                        nc.vector.tensor_add(vsum[:], vsum[:], v[:])
                        nc.scalar.copy(out=vsumh[:], in_=vsum[:])

    nc.compile()
    return nc


def _prepare_in_maps(inputs):
    x = np.asarray(inputs["x"], np.float32)
    weight = np.asarray(inputs["weight"], np.float32)
    wv, wvj, s1, s1f = _build_host_constants(weight)
    in_maps = []
    for core in range(NCORES):
        xl = x[core * BL:(core + 1) * BL]
        in_maps.append(_per_core_inputs(xl, wv, wvj, s1, s1f))
    return in_maps


def kernel(x, weight):
    from concourse.bass_utils import run_bass_kernel_spmd

    if "nc" not in _CACHE:
        _CACHE["nc"] = _build_program()
    nc = _CACHE["nc"]

    in_maps = _prepare_in_maps({"x": x, "weight": weight})
    res = run_bass_kernel_spmd(nc, in_maps, core_ids=list(range(NCORES)))
    _CACHE["last_results"] = res

    out = np.empty((B, O, DOUT), np.float32)
    for core in range(NCORES):
        oc = res.results[core]["out"]              # [BL, O, 8, 2]
        out[core * BL:(core + 1) * BL] = oc.reshape(BL, O, DOUT)
    return out


# revision 5
# speedup vs baseline: 1.0029x; 1.0029x over previous
"""DenseCapsule dynamic-routing kernel v2 for 8 Trainium2 NeuronCores.

Problem: x [256,1152,8] f32, weight [10,1152,16,8] f32 ->
  x_hat = einsum('oidc,bic->boid', weight, x)
  3 rounds of routing-by-agreement (softmax over o, squash over d)
  output [256, 10, 16] f32.

Layout (batch-parallel, 32 samples/core), i = kk*4 + g with g in [0,4):
  - Partition dim p = (b, g) = b*4+g (128 partitions).
  - Phase 1 matmul per kk (all at partition base 0, contraction 32):
      stationary XS[kk][(g,c), (b,g')] = delta_{g,g'} x[b, 4kk+g, c]
      moving     WV[kk][(g,c), (o,d)]  = W[o, 4kk+g, d, c]
    -> PSUM [(b,g), (o,d)] = x_hat[b, o, 4kk+g, d].
    Weight is NOT inflated (2.95 MB); x is inflated 4x (2.36 MB).
  - XH SBUF tile [128, KK, O, 8, 2] fp16 (d split 8x2 so the e-broadcast
    in the m-pass keeps unit innermost stride => DVE 2x mode).
  - t=0 capsule sums: PE-accumulated S1 matmuls over XH kk-triplets
    (S1 = delta_{b,b'} sums g within b, replicated over g').
  - Routing per t in kk-chunks: z = XH*vsum (1 TT), L = sum_d z
    (fp16 reduce), e = exp(L) on ACT engine written twice (pairs),
    Z = tree-sum over o, e' = e * (1/Z), m = XH*e' (per-o TT,
    pair-broadcast), s = S1-supers over m (480-col moving, LDW hidden).
  - squash entirely on [*, O, 8, 2] tiles (d in free dim, no PE).
"""

import sys

for _p in ("/opt/trn_rl_repo",):
    if _p not in sys.path:
        sys.path.insert(0, _p)

import numpy as np

B, I, DIN, O, DOUT = 256, 1152, 8, 10, 16
NCORES = 8
BL = B // NCORES          # 32 samples per core
G = 4                     # i's per phase-1 contraction block
KK = I // G               # 288 kk blocks
KC = 96                   # kk per routing chunk
NJ = (I * DIN) // 128     # 72 s0 chunks of 128 (i,c) rows
NCH = KK // KC            # routing chunks per iteration
TR = 3                    # kk per s-reduce matmul (3*160=480 <= 512)
ND = 12                   # input DMA chunks
KD = KK // ND             # kk per DMA chunk
EPS = 1e-8

_CACHE = {}


def _build_host_constants(weight):
    w5 = weight.reshape(O, KK, G, DOUT, DIN)           # [o,kk,g,d,c]
    wkgc = w5.transpose(1, 2, 4, 0, 3)                 # [kk,g,c,o,d]
    # wv[(g,c), kk, (o,d)] -> [32, KK, 160]
    wv = np.ascontiguousarray(
        wkgc.reshape(KK, 32, O * DOUT).transpose(1, 0, 2)).astype(np.float16)
    # wvj[(i,c) % 128, j, (o,d)] -> [128, NJ, 160]
    wvj = np.ascontiguousarray(
        wkgc.reshape(NJ, 128, O * DOUT).transpose(1, 0, 2)).astype(np.float16)

    bi = np.arange(128) // G
    s1 = (bi[:, None] == bi[None, :]).astype(np.float16)     # [128,128]
    s1f = (bi[:, None] == np.arange(BL)[None, :]).astype(np.float16)
    return wv, wvj, s1, s1f


def _per_core_inputs(xl, wv, wvj, s1, s1f):
    """xl: [BL, I, DIN] fp32 slice for this core."""
    x4 = xl.reshape(BL, KK, G, DIN).astype(np.float16)  # [b,kk,g,c]
    # xs[kk, (g,c), (b,g')] = delta_{g,g'} x[b, 4kk+g, c]
    xs = np.zeros((KK, G, DIN, BL, G), np.float16)      # [kk,g,c,b,g']
    xkcb = x4.transpose(1, 2, 3, 0)                     # [kk,g,c,b]
    for g in range(G):
        xs[:, g, :, :, g] = xkcb[:, g, :, :]
    xs = np.ascontiguousarray(
        xs.reshape(KK, 32, 128).transpose(1, 0, 2))     # [32, KK, 128]

    # xt4[(i,c), (b,g)] = x[b, i, c]
    xt = xl.reshape(BL, I * DIN).T.astype(np.float16)   # [(i,c), b]
    xt4 = np.repeat(xt, G, axis=1)                      # [(i,c), (b,g)]
    xt4 = np.ascontiguousarray(
        xt4.reshape(NJ, 128, 128).transpose(1, 0, 2))   # [128, NJ, 128]
    return {"xs": xs, "wv": wv, "wvj": wvj, "s1": s1, "s1f": s1f,
            "xt4": xt4}


def _squash(nc, small, s_ap, scale, f32, AX, ALU, NP):
    """squash(s*scale) on [NP, O, 8, 2] fp32; returns fp32 tile."""
    s_sb = small.tile([NP, O, 8, 2], f32, tag=f"sq_s{NP}")
    nc.scalar.mul(out=s_sb[:], in_=s_ap, mul=float(scale))
    sq = small.tile([NP, O, 8, 2], f32, tag=f"sq_sq{NP}")
    nc.vector.tensor_mul(sq[:], s_sb[:], s_sb[:])
    m2 = small.tile([NP, O, 1, 1], f32, tag=f"sq_m2{NP}")
    nc.vector.tensor_reduce(out=m2[:], in_=sq[:], axis=AX.XY, op=ALU.add)
    rt = small.tile([NP, O, 1, 1], f32, tag=f"sq_rt{NP}")
    nc.scalar.sqrt(out=rt[:], in_=m2[:])            # sqrt(mag2)
    nc.vector.tensor_scalar_add(rt[:], rt[:], EPS)
    den = small.tile([NP, O, 1, 1], f32, tag=f"sq_den{NP}")
    nc.scalar.add(out=den[:], in_=m2[:], add=1.0)   # 1 + mag2
    nc.vector.tensor_mul(den[:], den[:], rt[:])
    nc.vector.reciprocal_approx_fast(out=den[:, :, 0, 0], in_=den[:, :, 0, 0])
    fac = small.tile([NP, O, 1, 1], f32, tag=f"sq_fac{NP}")
    nc.vector.tensor_mul(fac[:], m2[:], den[:])
    v = small.tile([NP, O, 8, 2], f32, tag=f"sq_v{NP}")
    nc.vector.tensor_mul(v[:], s_sb[:], fac[:].broadcast_to((NP, O, 8, 2)))
    return v


def _build_program():
    import concourse.tile as tile
    from concourse import bacc, mybir

    f16 = mybir.dt.float16
    f32 = mybir.dt.float32
    AF = mybir.ActivationFunctionType
    AX = mybir.AxisListType
    ALU = mybir.AluOpType

    nc = bacc.Bacc(
        "TRN2",
        target_bir_lowering=False,
        debug=False,
        enable_asserts=False,
        num_devices=NCORES,
    )

    xs_d = nc.dram_tensor("xs", [32, KK, 128], f16, kind="ExternalInput")
    wv_d = nc.dram_tensor("wv", [32, KK, O * DOUT], f16, kind="ExternalInput")
    wvj_d = nc.dram_tensor("wvj", [128, NJ, O * DOUT], f16, kind="ExternalInput")
    xt4_d = nc.dram_tensor("xt4", [128, NJ, 128], f16, kind="ExternalInput")
    s1_d = nc.dram_tensor("s1", [128, 128], f16, kind="ExternalInput")
    s1f_d = nc.dram_tensor("s1f", [128, BL], f16, kind="ExternalInput")
    out_d = nc.dram_tensor("out", [BL, O, 8, 2], f32, kind="ExternalOutput")

    with tile.TileContext(nc) as tc:
        with (
            tc.tile_pool(name="const", bufs=1) as const,
            tc.tile_pool(name="xhp", bufs=1) as xhp,
            tc.tile_pool(name="acc", bufs=1) as acc,
            tc.tile_pool(name="small", bufs=1) as small,
            tc.tile_pool(name="zmp", bufs=2) as zmp,
            tc.tile_pool(name="spsum", bufs=1, space="PSUM") as spsum,
        ):
            s1_sb = const.tile([128, 128], f16)
            nc.gpsimd.dma_start(out=s1_sb[:], in_=s1_d.ap())
            s1f_sb = const.tile([128, BL], f16)
            nc.gpsimd.dma_start(out=s1f_sb[:], in_=s1f_d.ap())

            # x_hat, p=(b,g), free (kk, o, dd, r) with d = dd*2+r
            xh = xhp.tile([128, KK, O, 8, 2], f16)
            vsumh = acc.tile([128, O, 8, 2], f16)
            vsum = acc.tile([128, O, 8, 2], f32)

            # ---- Phase 1: x_hat + direct t=0 sums -----------------------
            JD = NJ // ND
            s0sp = spsum.tile([128, O, 8, 2], f32, tag="sp128")
            with (
                tc.tile_pool(name="wpool", bufs=2) as wpool,
                tc.tile_pool(name="xspool", bufs=2) as xspool,
                tc.tile_pool(name="wjpool", bufs=2) as wjpool,
                tc.tile_pool(name="xtpool", bufs=2) as xtpool,
                tc.tile_pool(name="ppsum", bufs=4, space="PSUM") as ppsum,
            ):
                NDS = 4                # dc's carrying the s0 work
                JD2 = NJ // NDS
                for dc in range(ND):
                    wck = wpool.tile([32, KD, O * DOUT], f16)
                    nc.gpsimd.dma_start(
                        out=wck[:], in_=wv_d.ap()[:, dc * KD:(dc + 1) * KD])
                    xsk = xspool.tile([32, KD, 128], f16)
                    nc.gpsimd.dma_start(
                        out=xsk[:], in_=xs_d.ap()[:, dc * KD:(dc + 1) * KD])
                    if dc < NDS:
                        for sub in range(JD2 // JD):
                            j0 = dc * JD2 + sub * JD
                            wjk = wjpool.tile([128, JD, O * DOUT], f16)
                            nc.gpsimd.dma_start(
                                out=wjk[:], in_=wvj_d.ap()[:, j0:j0 + JD])
                            xtk = xtpool.tile([128, JD, 128], f16)
                            nc.gpsimd.dma_start(
                                out=xtk[:], in_=xt4_d.ap()[:, j0:j0 + JD])
                            for jj in range(JD):
                                j = j0 + jj
                                # s0 = sum_(i,c) x*W : accumulate over j
                                nc.tensor.matmul(
                                    s0sp[:],
                                    lhsT=xtk[:, jj, :],
                                    rhs=wjk[:, jj, :],
                                    start=(j == 0),
                                    stop=(j == NJ - 1),
                                )
                    for s in range(KD // TR):
                        pt = ppsum.tile([128, TR, O, 8, 2], f32)
                        for r in range(TR):
                            nc.tensor.matmul(
                                pt[:, r],
                                lhsT=xsk[:, s * TR + r, :],
                                rhs=wck[:, s * TR + r, :],
                                start=True,
                                stop=True,
                            )
                        kk0 = dc * KD + s * TR
                        dst = xh[:, kk0:kk0 + TR]
                        if s % 6 == 0:
                            nc.vector.tensor_copy(out=dst, in_=pt[:])
                        else:
                            nc.scalar.copy(out=dst, in_=pt[:])
                    if dc == NDS - 1:
                        # ---- t = 0: uniform c = 1/10 (early) ------------
                        v = _squash(nc, small, s0sp[:], 1.0 / O,
                                    f32, AX, ALU, 128)
                        nc.vector.tensor_copy(out=vsum[:], in_=v[:])
                        nc.scalar.copy(out=vsumh[:], in_=vsum[:])

            # ---- t = 1, 2 ------------------------------------------------
            with nc.allow_low_precision(reason="logits/softmax in fp16"):
                for t in (1, 2):
                    final = t == 2
                    sS = s1f_sb if final else s1_sb
                    NP = BL if final else 128
                    sp = spsum.tile([NP, TR, O, 8, 2], f32, tag=f"tsp{NP}")
                    for ch in range(NCH):
                        k0 = ch * KC
                        zm = zmp.tile([128, KC, O, 8, 2], f16, tag="zm")
                        # z = XH * vsum  (vsum bcast over kk)
                        nc.vector.tensor_mul(
                            zm[:],
                            xh[:, k0:k0 + KC],
                            vsumh[:].unsqueeze(1)
                            .broadcast_to((128, KC, O, 8, 2)),
                        )
                        # L = sum_d z -> [128, KC, O] fp16 (in-place pair
                        # tree in zm; all levels unit-stride => DVE 2x)
                        nc.vector.tensor_add(
                            zm[:, :, :, 0:4], zm[:, :, :, 0:4], zm[:, :, :, 4:8])
                        nc.vector.tensor_add(
                            zm[:, :, :, 0:2], zm[:, :, :, 0:2], zm[:, :, :, 2:4])
                        nc.vector.tensor_add(
                            zm[:, :, :, 0:1], zm[:, :, :, 0:1], zm[:, :, :, 1:2])
                        L = small.tile([128, KC, O], f16, tag=f"L{ch % 2}")
                        nc.vector.tensor_add(
                            L[:], zm[:, :, :, 0, 0], zm[:, :, :, 0, 1])
                        # e = exp(L), written twice (pairs)
                        e2 = small.tile([128, KC, O, 2], f16, tag=f"e2{ch % 2}")
                        nc.scalar.activation(
                            out=e2[:, :, :, 0], in_=L[:], func=AF.Exp)
                        nc.scalar.activation(
                            out=e2[:, :, :, 1], in_=L[:], func=AF.Exp)
                        # Z = sum_o e (pairs tree)
                        t5 = small.tile([128, KC, 5, 2], f16, tag="t5")
                        nc.vector.tensor_add(t5[:], e2[:, :, 0:5], e2[:, :, 5:10])
                        u2 = small.tile([128, KC, 2, 2], f16, tag="u2")
                        nc.vector.tensor_add(u2[:], t5[:, :, 0:2], t5[:, :, 2:4])
                        zden = small.tile([128, KC, 1, 2], f32, tag="zden")
                        nc.vector.tensor_add(zden[:], u2[:, :, 0:1], u2[:, :, 1:2])
                        nc.vector.tensor_add(zden[:], zden[:], t5[:, :, 4:5])
                        nc.vector.reciprocal_approx_fast(
                            out=zden[:, :, 0, :], in_=zden[:, :, 0, :])
                        zinv = small.tile([128, KC, 1, 2], f16, tag="zinv")
                        nc.vector.tensor_copy(out=zinv[:], in_=zden[:])
                        # e' = e * (1/Z)  (bcast over o)
                        nc.vector.tensor_mul(
                            e2[:], e2[:], zinv[:].broadcast_to((128, KC, O, 2)))
                        # m = XH * e'  (pair-bcast over dd) -- per o
                        for o in range(O):
                            nc.vector.tensor_mul(
                                zm[:, :, o],
                                xh[:, k0:k0 + KC, o],
                                e2[:, :, o].unsqueeze(2)
                                .broadcast_to((128, KC, 8, 2)),
                            )
                        # s += sum_{kk,g} m : PE accumulation, kk-triplets
                        for s in range(KC // TR):
                            nc.tensor.matmul(
                                sp[:],
                                lhsT=sS[:],
                                rhs=zm[:, TR * s:TR * s + TR],
                                start=(ch == 0 and s == 0),
                                stop=(ch == NCH - 1 and s == KC // TR - 1),
                            )
                    stot = small.tile([NP, O, 8, 2], f32, tag=f"stot{NP}")
                    nc.scalar.copy(out=stot[:], in_=sp[:, 0])
                    nc.vector.tensor_add(stot[:], stot[:], sp[:, 1])
                    nc.vector.tensor_add(stot[:], stot[:], sp[:, 2])
                    v = _squash(nc, small, stot[:], 1.0, f32, AX, ALU, NP)
                    if final:
                        nc.gpsimd.dma_start(out=out_d.ap(), in_=v[:])
                    else:
                        nc.vector.tensor_add(vsum[:], vsum[:], v[:])
                        nc.scalar.copy(out=vsumh[:], in_=vsum[:])

    nc.compile()
    return nc


def _prepare_in_maps(inputs):
    x = np.asarray(inputs["x"], np.float32)
    weight = np.asarray(inputs["weight"], np.float32)
    wv, wvj, s1, s1f = _build_host_constants(weight)
    in_maps = []
    for core in range(NCORES):
        xl = x[core * BL:(core + 1) * BL]
        in_maps.append(_per_core_inputs(xl, wv, wvj, s1, s1f))
    return in_maps


def kernel(x, weight):
    from concourse.bass_utils import run_bass_kernel_spmd

    if "nc" not in _CACHE:
        _CACHE["nc"] = _build_program()
    nc = _CACHE["nc"]

    in_maps = _prepare_in_maps({"x": x, "weight": weight})
    res = run_bass_kernel_spmd(nc, in_maps, core_ids=list(range(NCORES)))
    _CACHE["last_results"] = res

    out = np.empty((B, O, DOUT), np.float32)
    for core in range(NCORES):
        oc = res.results[core]["out"]              # [BL, O, 8, 2]
        out[core * BL:(core + 1) * BL] = oc.reshape(BL, O, DOUT)
    return out
